# revision 4
# baseline (speedup 1.0000x reference)
"""Trainium2 Bass kernel for a dense transformer block (single-head attn + MLP).

Sharding: 8 cores; core c handles batch b=c//2, query-half h=c%2.
Each core computes K,V for all 2048 tokens of its batch (no collectives).
Host permutes tokens so each core's query tokens are always columns 0..1023
of its transposed input (SPMD uniform program).

Layout: activations kept transposed [C, T] (channels on partitions) so every
matmul feeds the PE directly.  LN stats via ones-matmuls on PE; per-token row
broadcasts via K=1 outer-product matmuls; softmax without max subtraction
(logits are ~N(0, 0.4^2): verified small); softmax denominator folded into
the y-eviction.  All matmuls in float32r (FP22, full PE rate at N>=256).
"""

import numpy as np
import concourse.bass as bass
import concourse.mybir as mybir
import concourse.tile as tile
from concourse import bacc
from concourse.bass_utils import run_bass_kernel_spmd

F32 = mybir.dt.float32
F32R = mybir.dt.float32r
AF = mybir.ActivationFunctionType
ALU = mybir.AluOpType

P = 128
C = 1024        # n_embd
T = 2048        # key tokens per batch
TQ = 1024       # query tokens per core
H = 4096        # mlp hidden
CK = C // P     # 8
HK = H // P     # 32
S = T // P      # 16 key tiles
NCH = 512       # matmul moving-dim chunk
EPS = 1e-5
ATT_SCALE = 1.0 / 32.0   # 1/sqrt(C)

N_CORES = 8


def _build():
    nc = bacc.Bacc()

    xT = nc.declare_dram_parameter("xT", [C, T], F32R, isOutput=False)
    w1qk = nc.declare_dram_parameter("w1qk", [2 * CK, P, C], F32R, isOutput=False)
    w1v = nc.declare_dram_parameter("w1v", [CK, P, C], F32R, isOutput=False)
    wp = nc.declare_dram_parameter("wp", [CK, P, C], F32R, isOutput=False)
    w2 = nc.declare_dram_parameter("w2", [HK, P, C], F32R, isOutput=False)
    wm = nc.declare_dram_parameter("wm", [CK, P, H], F32R, isOutput=False)
    c1q = nc.declare_dram_parameter("c1q", [CK, P], F32, isOutput=False)
    c1k = nc.declare_dram_parameter("c1k", [CK, P], F32, isOutput=False)
    c1vb = nc.declare_dram_parameter("c1vb", [P, C], F32, isOutput=False)
    bp = nc.declare_dram_parameter("bp", [CK, P], F32, isOutput=False)
    c2 = nc.declare_dram_parameter("c2", [HK, P], F32, isOutput=False)
    bm = nc.declare_dram_parameter("bm", [CK, P], F32, isOutput=False)
    onc = nc.declare_dram_parameter("onc", [P, 1], F32R, isOutput=False)
    onr = nc.declare_dram_parameter("onr", [1, P], F32R, isOutput=False)
    out_t = nc.declare_dram_parameter("out_t", [C, TQ], F32, isOutput=True)

    ktd = nc.dram_tensor("ktd", [CK, P, T], F32R)
    vd = nc.dram_tensor("vd", [S, P, C], F32R)
    attd = nc.dram_tensor("attd", [S, P, TQ], F32R)

    xT3 = xT.rearrange("(k p) t -> p k t", p=P)

    with tile.TileContext(nc) as tc:
        with (
            tc.tile_pool(name="glob", bufs=1) as gp,
            tc.tile_pool(name="ps", bufs=8, space="PSUM") as pp,
        ):
            def pst(pdim=P):
                return pp.tile([pdim, NCH], F32, tag="ps", name="ps")

            ones_col = gp.tile([P, 1], F32R)
            nc.sync.dma_start(ones_col[:], onc[:])
            ones_row = gp.tile([1, P], F32R)
            nc.sync.dma_start(ones_row[:], onr[:])
            c1q_t = gp.tile([P, CK], F32)
            nc.sync.dma_start(c1q_t[:], c1q.rearrange("j p -> p j"))
            c1k_t = gp.tile([P, CK], F32)
            nc.sync.dma_start(c1k_t[:], c1k.rearrange("j p -> p j"))
            bp_t = gp.tile([P, CK], F32)
            nc.sync.dma_start(bp_t[:], bp.rearrange("j p -> p j"))
            bm_t = gp.tile([P, CK], F32)
            nc.sync.dma_start(bm_t[:], bm.rearrange("j p -> p j"))
            c2_t = gp.tile([P, HK], F32)
            nc.sync.dma_start(c2_t[:], c2.rearrange("j p -> p j"))
            c1v_t = gp.tile([P, C], F32)
            nc.sync.dma_start(c1v_t[:], c1vb[:])
            recip_b = gp.tile([P, TQ], F32)
            eps_col = gp.tile([P, 1], F32)
            nc.vector.memset(eps_col[:], EPS)

            def ln_stats_and_norm(src3, n_ck, width, r_b, mur_b, rbase, xh):
                """LN over channels of a transposed activation tile.
                src3: [P, n_ck, >=width] SBUF tile (local cols 0..width);
                writes normalized f32r into xh[:, :, 0:width]; fills
                r_b/mur_b[:, rbase:rbase+width]."""
                for sub in range(width // NCH):
                    lo = sub * NCH
                    mu_ps = pst(1)
                    s2_ps = pst(1)
                    for k in range(n_ck):
                        nc.tensor.matmul(mu_ps[:], ones_col[:],
                                         src3[:, k, lo:lo + NCH],
                                         start=(k == 0), stop=(k == n_ck - 1))
                    for k in range(n_ck):
                        sq = gp.tile([P, NCH], F32R, tag="sq", bufs=3)
                        nc.scalar.activation(
                            sq[:], src3[:, k, lo:lo + NCH].bitcast(F32),
                            AF.Square)
                        nc.tensor.matmul(s2_ps[:], ones_col[:], sq[:],
                                         start=(k == 0), stop=(k == n_ck - 1))
                    mu_sb = gp.tile([1, NCH], F32R, tag="murow", bufs=2)
                    nc.scalar.activation(mu_sb[:], mu_ps[:], AF.Copy, scale=1.0 / C)
                    musq = gp.tile([1, NCH], F32, tag="musq", bufs=2)
                    nc.scalar.activation(musq[:], mu_ps[:], AF.Square, scale=1.0 / C)
                    var_sb = gp.tile([1, NCH], F32R, tag="varrow", bufs=2)
                    nc.vector.scalar_tensor_tensor(
                        var_sb[:], s2_ps[:], 1.0 / C, musq[:],
                        op0=ALU.mult, op1=ALU.subtract)
                    vb_ps = pst()
                    nc.tensor.matmul(vb_ps[:], ones_row[:], var_sb[:],
                                     start=True, stop=True)
                    mb_ps = pst()
                    nc.tensor.matmul(mb_ps[:], ones_row[:], mu_sb[:],
                                     start=True, stop=True)
                    rbs = r_b[:, rbase + lo:rbase + lo + NCH]
                    nc.scalar.activation(rbs, vb_ps[:], AF.Sqrt, bias=eps_col[:])
                    nc.vector.reciprocal(rbs, rbs)
                    nc.vector.tensor_mul(
                        mur_b[:, rbase + lo:rbase + lo + NCH], mb_ps[:], rbs)
                for k in range(n_ck):
                    dst = xh[:, k, 0:width]
                    nc.vector.tensor_mul(
                        dst, src3[:, k, 0:width].bitcast(F32),
                        r_b[:, rbase:rbase + width])
                    nc.vector.tensor_sub(dst, dst.bitcast(F32),
                                         mur_b[:, rbase:rbase + width])

            # ============ phase A (LN1 + QKV) and B (scores) ============
            with tc.tile_pool(name="ab", bufs=1) as abp:
                qT = abp.tile([P, CK, TQ], F32R)
                with tc.tile_pool(name="pa", bufs=1) as pa:
                    r_b = gp.tile([P, T], F32, tag="rb")
                    mur_b = gp.tile([P, T], F32, tag="murb")
                    w1v_sb = pa.tile([P, CK, C], F32R, tag="w1v")
                    nc.sync.dma_start(w1v_sb[:], w1v.rearrange("k p c -> p k c"))
                    for half in range(2):          # 1024-token chunks over T
                        tlo, thi = half * TQ, (half + 1) * TQ
                        xt = pa.tile([P, CK, TQ], F32R, tag="xt", bufs=1)
                        nc.sync.dma_start(xt[:], xT3[:, :, tlo:thi])
                        xh = pa.tile([P, CK, TQ], F32R, tag="xh", bufs=1)
                        ln_stats_and_norm(xt, CK, TQ, r_b, mur_b, tlo, xh)
                        # q (half 0 only) and k projections
                        jlist = list(range(CK, 2 * CK))
                        if half == 0:
                            jlist = list(range(CK)) + jlist
                        for j in jlist:
                            wblk = pa.tile([P, C], F32R, tag="wqk", bufs=2)
                            nc.sync.dma_start(wblk[:], w1qk[j])
                            for sub in range(TQ // NCH):
                                o_ps = pst()
                                for k in range(CK):
                                    nc.tensor.matmul(
                                        o_ps[:], wblk[:, k * P:(k + 1) * P],
                                        xh[:, k, sub * NCH:(sub + 1) * NCH],
                                        start=(k == 0), stop=(k == CK - 1))
                                if j < CK:
                                    nc.vector.tensor_scalar(
                                        qT[:, j, sub * NCH:(sub + 1) * NCH],
                                        o_ps[:], c1q_t[:, j:j + 1], None,
                                        op0=ALU.add)
                                else:
                                    kt_sb = pa.tile([P, NCH], F32R, tag="ktev",
                                                    bufs=3)
                                    nc.vector.tensor_scalar(
                                        kt_sb[:], o_ps[:],
                                        c1k_t[:, j - CK:j - CK + 1], None,
                                        op0=ALU.add)
                                    nc.sync.dma_start(
                                        ktd[j - CK, :,
                                            tlo + sub * NCH:tlo + (sub + 1) * NCH],
                                        kt_sb[:])
                        # v projection (natural layout, s-tiles of this half)
                        for sl in range(TQ // P):
                            s_idx = half * (TQ // P) + sl
                            for cc in range(C // NCH):
                                v_ps = pst()
                                for k in range(CK):
                                    nc.tensor.matmul(
                                        v_ps[:],
                                        xh[:, k, sl * P:(sl + 1) * P],
                                        w1v_sb[:, k, cc * NCH:(cc + 1) * NCH],
                                        start=(k == 0), stop=(k == CK - 1))
                                v_sb = pa.tile([P, NCH], F32R, tag="vev", bufs=3)
                                nc.vector.tensor_add(
                                    v_sb[:], v_ps[:],
                                    c1v_t[:, cc * NCH:(cc + 1) * NCH])
                                nc.sync.dma_start(
                                    vd[s_idx, :, cc * NCH:(cc + 1) * NCH],
                                    v_sb[:])
                # -------- phase B: scores + exp + denominators --------
                sums_ps = []
                with tc.tile_pool(name="pb", bufs=1) as pb:
                    for sub in range(TQ // NCH):
                        sums_ps.append(pst(1))
                    for s in range(S):
                        kt = pb.tile([P, CK, P], F32R, tag="kt", bufs=3)
                        nc.sync.dma_start(
                            kt[:],
                            ktd[:, :, s * P:(s + 1) * P].rearrange(
                                "k p s -> p k s"))
                        for sub in range(TQ // NCH):
                            a_ps = pst()
                            for k in range(CK):
                                nc.tensor.matmul(
                                    a_ps[:], kt[:, k, :],
                                    qT[:, k, sub * NCH:(sub + 1) * NCH],
                                    start=(k == 0), stop=(k == CK - 1))
                            ae = pb.tile([P, NCH], F32R, tag="attev", bufs=3)
                            nc.scalar.activation(ae[:], a_ps[:], AF.Exp,
                                                 scale=ATT_SCALE)
                            nc.sync.dma_start(
                                attd[s, :, sub * NCH:(sub + 1) * NCH], ae[:])
                            nc.tensor.matmul(sums_ps[sub][:], ones_col[:],
                                             ae[:], start=(s == 0),
                                             stop=(s == S - 1))

            # ============ phase C (y + proj + residual) and D (MLP) ============
            with tc.tile_pool(name="cd", bufs=1) as cdp:
                x2T = cdp.tile([P, CK, TQ], F32R)
                with tc.tile_pool(name="pc", bufs=1) as pc:
                    # reciprocal of softmax denominators, broadcast
                    for sub in range(TQ // NCH):
                        srow = pc.tile([1, NCH], F32, tag="srow", bufs=2)
                        nc.scalar.activation(srow[:], sums_ps[sub][:], AF.Copy)
                        nc.vector.reciprocal(srow[:], srow[:])
                        srow_r = pc.tile([1, NCH], F32R, tag="srowr", bufs=2)
                        nc.scalar.activation(srow_r[:], srow[:], AF.Copy)
                        rb_ps = pst()
                        nc.tensor.matmul(rb_ps[:], ones_row[:], srow_r[:],
                                         start=True, stop=True)
                        nc.vector.tensor_copy(
                            recip_b[:, sub * NCH:(sub + 1) * NCH], rb_ps[:])
                    yT = pc.tile([P, CK, TQ], F32R, tag="yT")
                    for sub in range(TQ // NCH):
                        y_ps = [pst() for _ in range(CK)]
                        for s in range(S):
                            va = pc.tile([P, C], F32R, tag="va", bufs=3)
                            nc.sync.dma_start(va[:], vd[s])
                            ar = pc.tile([P, NCH], F32R, tag="ar", bufs=3)
                            nc.sync.dma_start(
                                ar[:], attd[s, :, sub * NCH:(sub + 1) * NCH])
                            for cti in range(CK):
                                nc.tensor.matmul(
                                    y_ps[cti][:], va[:, cti * P:(cti + 1) * P],
                                    ar[:], start=(s == 0), stop=(s == S - 1))
                        for cti in range(CK):
                            nc.vector.tensor_mul(
                                yT[:, cti, sub * NCH:(sub + 1) * NCH],
                                y_ps[cti][:],
                                recip_b[:, sub * NCH:(sub + 1) * NCH])
                    # proj + bias + residual -> x2T
                    for j in range(CK):
                        wpb = pc.tile([P, C], F32R, tag="wpb", bufs=2)
                        nc.sync.dma_start(wpb[:], wp[j])
                        for sub in range(TQ // NCH):
                            z_ps = pst()
                            for k in range(CK):
                                nc.tensor.matmul(
                                    z_ps[:], wpb[:, k * P:(k + 1) * P],
                                    yT[:, k, sub * NCH:(sub + 1) * NCH],
                                    start=(k == 0), stop=(k == CK - 1))
                            xq = pc.tile([P, NCH], F32R, tag="xq", bufs=3)
                            nc.sync.dma_start(
                                xq[:],
                                xT3[:, j, sub * NCH:(sub + 1) * NCH])
                            nc.vector.scalar_tensor_tensor(
                                x2T[:, j, sub * NCH:(sub + 1) * NCH],
                                z_ps[:], bp_t[:, j:j + 1], xq[:].bitcast(F32),
                                op0=ALU.add, op1=ALU.add)
                # -------- phase D: LN2 + MLP + final residual --------
                with tc.tile_pool(name="pd", bufs=1) as pd:
                    r2_b = gp.tile([P, TQ], F32, tag="rb")
                    mur2_b = gp.tile([P, TQ], F32, tag="murb")
                    xh2 = pd.tile([P, CK, TQ], F32R, tag="xh2")
                    ln_stats_and_norm(x2T, CK, TQ, r2_b, mur2_b, 0, xh2)
                    for sub in range(TQ // NCH):
                        gel = pd.tile([P, HK, NCH], F32R, tag="gel", bufs=1)
                        for jh in range(HK):
                            wb2 = pd.tile([P, C], F32R, tag="wb2", bufs=2)
                            nc.sync.dma_start(wb2[:], w2[jh])
                            m_ps = pst()
                            for k in range(CK):
                                nc.tensor.matmul(
                                    m_ps[:], wb2[:, k * P:(k + 1) * P],
                                    xh2[:, k, sub * NCH:(sub + 1) * NCH],
                                    start=(k == 0), stop=(k == CK - 1))
                            nc.scalar.activation(
                                gel[:, jh, :], m_ps[:], AF.Gelu_apprx_tanh,
                                bias=c2_t[:, jh:jh + 1])
                        for j in range(CK):
                            o_ps = pst()
                            for hh in range(4):
                                wmb = pd.tile([P, H // 4], F32R, tag="wmb",
                                              bufs=2)
                                nc.sync.dma_start(
                                    wmb[:],
                                    wm[j, :, hh * (H // 4):(hh + 1) * (H // 4)])
                                for kk in range(HK // 4):
                                    k = hh * (HK // 4) + kk
                                    nc.tensor.matmul(
                                        o_ps[:], wmb[:, kk * P:(kk + 1) * P],
                                        gel[:, k, :],
                                        start=(k == 0), stop=(k == HK - 1))
                            o_sb = pd.tile([P, NCH], F32, tag="oev", bufs=3)
                            nc.vector.scalar_tensor_tensor(
                                o_sb[:], o_ps[:], bm_t[:, j:j + 1],
                                x2T[:, j, sub * NCH:(sub + 1) * NCH].bitcast(F32),
                                op0=ALU.add, op1=ALU.add)
                            nc.sync.dma_start(
                                out_t[j * P:(j + 1) * P,
                                      sub * NCH:(sub + 1) * NCH], o_sb[:])
    nc.finalize()
    return nc


_prog = None


def _get_prog():
    global _prog
    if _prog is None:
        _prog = _build()
    return _prog


def _pack_weights(ln1_g, ln1_b, w_attn, b_attn, w_proj, b_proj,
                  ln2_g, ln2_b, w_fc, b_fc, w_mlp_proj, b_mlp_proj):
    f = np.float32
    W1 = (ln1_g[:, None] * w_attn).astype(f)            # [C, 3C]
    c1 = (ln1_b @ w_attn + b_attn).astype(f)            # [3C]
    w1qk = np.ascontiguousarray(
        W1[:, :2 * C].reshape(CK, P, 2 * CK, P).transpose(2, 1, 0, 3)
        .reshape(2 * CK, P, C))
    w1v = np.ascontiguousarray(W1[:, 2 * C:].reshape(CK, P, C))
    wp_t = np.ascontiguousarray(
        w_proj.reshape(CK, P, CK, P).transpose(2, 1, 0, 3).reshape(CK, P, C))
    W2 = (ln2_g[:, None] * w_fc).astype(f)              # [C, H]
    c2v = (ln2_b @ w_fc + b_fc).astype(f)               # [H]
    w2_t = np.ascontiguousarray(
        W2.reshape(CK, P, HK, P).transpose(2, 1, 0, 3).reshape(HK, P, C))
    wm_t = np.ascontiguousarray(
        w_mlp_proj.astype(f).reshape(HK, P, CK, P).transpose(2, 1, 0, 3)
        .reshape(CK, P, H))
    return {
        "w1qk": w1qk,
        "w1v": w1v,
        "wp": wp_t,
        "w2": w2_t,
        "wm": wm_t,
        "c1q": np.ascontiguousarray(c1[:C].reshape(CK, P)),
        "c1k": np.ascontiguousarray(c1[C:2 * C].reshape(CK, P)),
        "c1vb": np.ascontiguousarray(
            np.broadcast_to(c1[2 * C:], (P, C)).astype(f)),
        "bp": np.ascontiguousarray(b_proj.astype(f).reshape(CK, P)),
        "c2": np.ascontiguousarray(c2v.reshape(HK, P)),
        "bm": np.ascontiguousarray(b_mlp_proj.astype(f).reshape(CK, P)),
        "onc": np.ones((P, 1), f),
        "onr": np.ones((1, P), f),
    }


def kernel(x, ln1_g, ln1_b, w_attn, b_attn, w_proj, b_proj,
           ln2_g, ln2_b, w_fc, b_fc, w_mlp_proj, b_mlp_proj,
           _trace=False):
    x = np.asarray(x, np.float32)
    shared = _pack_weights(
        np.asarray(ln1_g, np.float32), np.asarray(ln1_b, np.float32),
        np.asarray(w_attn, np.float32), np.asarray(b_attn, np.float32),
        np.asarray(w_proj, np.float32), np.asarray(b_proj, np.float32),
        np.asarray(ln2_g, np.float32), np.asarray(ln2_b, np.float32),
        np.asarray(w_fc, np.float32), np.asarray(b_fc, np.float32),
        np.asarray(w_mlp_proj, np.float32), np.asarray(b_mlp_proj, np.float32))

    in_maps = []
    for core in range(N_CORES):
        b, h = core // 2, core % 2
        xb = x[b]                                    # [T, C]
        mine = xb[h * TQ:(h + 1) * TQ]
        other = xb[(1 - h) * TQ:(2 - h) * TQ]
        xTc = np.ascontiguousarray(
            np.concatenate([mine, other], axis=0).T)  # [C, T]
        in_maps.append({"xT": xTc, **shared})

    nc = _get_prog()
    res = run_bass_kernel_spmd(nc, in_maps, list(range(N_CORES)),
                               trace=_trace)
    out = np.empty_like(x)
    for core in range(N_CORES):
        b, h = core // 2, core % 2
        out[b, h * TQ:(h + 1) * TQ] = res.results[core]["out_t"].T
    if _trace:
        kernel._last_exec_time_ns = res.exec_time_ns
        kernel._last_profile = res.profile_json
    return out
